# Initial kernel scaffold
#
"""Trainium2 Bass kernel for nn_FFTConv: y = tanh(Re(ifft(fft(u)*Ks)) + D*u).

Self-contained: builds constant tables with numpy, shards over 8 NeuronCores
(H-parallel: 32 channels/core), runs a Bass/Tile kernel per core via
run_bass_kernel_spmd, gathers the full output.

Algorithm (per core):
  Prologue:
    G[p,m] = 1/(1 - A_p * WL^m)           (P=64 poles x L=8192 freqs, on DVE/ACT)
    Ks[h]  = BC[h,:] @ G                  (TensorE, -> DRAM scratch, complex)
  Main loop over (h, b-group of 4):  2-stage matmul FFT, L = 128*64
    n = n1 + 128*n2 ; m = k2 + 64*k1
    Y1 = F64 @ u.reshape(64,128)          [k2, n1]
    Y2 = Y1 * T                           twiddle T[k2,n1] = WL^(n1*k2)
    X  = F128 @ Y2.T                      [k1, k2]  (PE transpose between)
    S  = X * Ks[h].reshape(128,64)
    Z1 = conj(F128) @ S                   [o2, k2]
    Z2 = Z1 * conj(TI)                    TI[o2,k2] = WL^(k2*o2)
    xo = Re(conj(F64) @ Z2.T)             [o1, o2]  (PE transpose between)
    y  = tanh(xo/L + D[h]*u)
"""
import os
import sys
import numpy as np

for p in ("/opt/trn_rl_repo", "/root/.axon_site/_ro/trn_rl_repo"):
    if os.path.isdir(p) and p not in sys.path:
        sys.path.append(p)

B, H, L, P = 16, 256, 8192, 64
NCORES = 8
HSH = H // NCORES          # 32 channels per core
GB = 4                     # b-group size (pairs per inner group)
NG = B // GB               # 4 groups per h
F32R = os.environ.get("KERNEL_F32R", "0") == "1"   # reduced-precision fast matmul mode
REPEAT = int(os.environ.get("KERNEL_REPEAT", "1"))  # repeat main loop (timing only)
MIDBUFS = int(os.environ.get("KERNEL_MIDBUFS", "2"))
IOBUFS = int(os.environ.get("KERNEL_IOBUFS", "3"))

_CACHE = {}


def _tables():
    a64 = np.arange(64)
    a128 = np.arange(128)
    th64 = 2 * np.pi * np.outer(a64, a64) / 64.0
    th128 = 2 * np.pi * np.outer(a128, a128) / 128.0
    thT = 2 * np.pi * np.outer(a64, a128) / L       # [k2, n1]
    thTI = 2 * np.pi * np.outer(a128, a64) / L      # [o2, k2]
    t = {
        "f64r": np.cos(th64), "f64i": -np.sin(th64),
        "f128r": np.cos(th128), "f128i": -np.sin(th128), "f128in": np.sin(th128),
        "tr": np.cos(thT), "ti": -np.sin(thT),
        # conj(TI) passed directly: re=cos, im=+sin
        "tir": np.cos(thTI), "tii": np.sin(thTI),
        "i64": np.eye(64), "i128": np.eye(128),
    }
    m = np.arange(L)
    cm = np.cos(2 * np.pi * m / L).reshape(2, 4096)
    sm = np.sin(2 * np.pi * m / L).reshape(2, 4096)
    # pre-replicated across 64 partitions per half: (128, 4096)
    t["cm"] = np.repeat(cm, 64, axis=0)
    t["sm"] = np.repeat(sm, 64, axis=0)
    return {k: v.astype(np.float32) for k, v in t.items()}


def _build(nc_mod):
    """Builds the Bass program (same program for all cores)."""
    bass, tile, mybir, bacc = nc_mod
    dt = mybir.dt
    f32 = dt.float32
    MMDT = dt.float32r if F32R else dt.float32

    def mdt(ap):
        return ap.bitcast(MMDT) if F32R else ap

    nc = bacc.Bacc("TRN2", target_bir_lowering=False, debug=False)
    AF = mybir.ActivationFunctionType
    OP = mybir.AluOpType

    # ---------------- DRAM parameters ----------------
    u_d = nc.declare_dram_parameter("u_sh", [B, HSH, L], f32, isOutput=False)
    y_d = nc.declare_dram_parameter("y_sh", [B, HSH, L], f32, isOutput=True)
    ar_d = nc.declare_dram_parameter("a_re", [2 * P, 1], f32, isOutput=False)
    ai_d = nc.declare_dram_parameter("a_im", [2 * P, 1], f32, isOutput=False)
    bcr_d = nc.declare_dram_parameter("bct_r", [P, HSH], f32, isOutput=False)
    bci_d = nc.declare_dram_parameter("bct_i", [P, HSH], f32, isOutput=False)
    bcin_d = nc.declare_dram_parameter("bct_i_neg", [P, HSH], f32, isOutput=False)
    d_d = nc.declare_dram_parameter("d_sh", [1, HSH], f32, isOutput=False)
    tbl_names = ["f64r", "f64i", "f128r", "f128i", "f128in",
                 "tr", "ti", "tir", "tii", "i64", "i128", "cm", "sm"]
    tbl_shapes = {"f64r": [64, 64], "f64i": [64, 64],
                  "f128r": [128, 128], "f128i": [128, 128], "f128in": [128, 128],
                  "tr": [64, 128], "ti": [64, 128],
                  "tir": [128, 64], "tii": [128, 64],
                  "i64": [64, 64], "i128": [128, 128],
                  "cm": [128, 4096], "sm": [128, 4096]}
    tbl_d = {n: nc.declare_dram_parameter(n, tbl_shapes[n], f32, isOutput=False) for n in tbl_names}

    ksr_d = nc.dram_tensor("ksr_scratch", [HSH, L], f32)
    ksi_d = nc.dram_tensor("ksi_scratch", [HSH, L], f32)

    with tile.TileContext(nc) as tc:
        with (
            tc.tile_pool(name="const", bufs=1) as cpool,
            tc.tile_pool(name="ks", bufs=2) as ksp,
        ):
            # ---------- load constants ----------
            tb = {}
            for n in tbl_names:
                if n in ("cm", "sm"):
                    continue
                tb[n] = cpool.tile(tbl_shapes[n], f32, tag=f"c_{n}", name=f"c_{n}")
                if n in ("f64r", "f64i", "f128r", "f128i", "f128in"):
                    nc.sync.dma_start(mdt(tb[n][:]), mdt(tbl_d[n][:]))
                else:
                    nc.sync.dma_start(tb[n][:], tbl_d[n][:])
            a_re = cpool.tile([128, 1], f32, tag="a_re")
            a_im = cpool.tile([128, 1], f32, tag="a_im")
            nc.sync.dma_start(a_re[:], ar_d[:])
            nc.sync.dma_start(a_im[:], ai_d[:])
            bct_r = cpool.tile([P, HSH], f32, tag="bct_r")
            bct_i = cpool.tile([P, HSH], f32, tag="bct_i")
            bct_in = cpool.tile([P, HSH], f32, tag="bct_in")
            nc.sync.dma_start(mdt(bct_r[:]), mdt(bcr_d[:]))
            nc.sync.dma_start(mdt(bct_i[:]), mdt(bci_d[:]))
            nc.sync.dma_start(mdt(bct_in[:]), mdt(bcin_d[:]))
            # D broadcast to 64 partitions
            d_b = cpool.tile([64, HSH], f32, tag="d_b")
            nc.sync.dma_start(d_b[:], d_d[:].broadcast_to([64, HSH]))

            prologue_pools = (
                tc.tile_pool(name="gwork", bufs=1),
                tc.tile_pool(name="psk", bufs=2, space=bass.MemorySpace.PSUM),
            )
            gpool = prologue_pools[0].__enter__()
            pskp = prologue_pools[1].__enter__()
            # 1 + |A|^2 per partition (stacked twice)
            one_a2 = cpool.tile([128, 1], f32, tag="one_a2")
            t_sq = gpool.tile([128, 1], f32, tag="g_sq")
            nc.scalar.activation(one_a2[:], a_re[:], AF.Square)
            nc.scalar.activation(t_sq[:], a_im[:], AF.Square)
            nc.vector.tensor_tensor(one_a2[:], one_a2[:], t_sq[:], OP.add)
            nc.vector.tensor_scalar_add(one_a2[:], one_a2[:], 1.0)

            # ---------- G = 1/(1 - A*WL^m), layout (128p=[p|p], 4096f) ----------
            ctab = gpool.tile([128, 4096], f32, tag="g_ctab")
            stab = gpool.tile([128, 4096], f32, tag="g_stab")
            nc.sync.dma_start(ctab[:], tbl_d["cm"][:])
            nc.sync.dma_start(stab[:], tbl_d["sm"][:])
            gq = gpool.tile([128, 4096], f32, tag="g_q")
            gt = gpool.tile([128, 4096], f32, tag="g_t")
            gdr = gpool.tile([128, 4096], f32, tag="g_dr")
            gn2 = gpool.tile([128, 4096], f32, tag="g_n2")
            g_r = gpool.tile([128, 4096], f32, tag="g_r")
            g_i = gpool.tile([128, 4096], f32, tag="g_i")
            # q = A_re*C + A_im*S
            nc.vector.tensor_scalar_mul(gq[:], ctab[:], a_re[:])
            nc.scalar.activation(gt[:], stab[:], AF.Identity, scale=a_im[:])
            nc.vector.tensor_tensor(gq[:], gq[:], gt[:], OP.add)
            # dr = 1 - q ; n2 = 1+|A|^2 - 2q ; rn = 1/n2
            nc.scalar.activation(gdr[:], gq[:], AF.Identity, scale=-1.0, bias=1.0)
            nc.vector.tensor_scalar(gn2[:], gq[:], -2.0, one_a2[:], OP.mult, OP.add)
            nc.vector.reciprocal(gn2[:], gn2[:])
            # di_n = A_im*C - A_re*S  (numerator of +Gi)
            g_t4 = gpool.tile([128, 4096], f32, tag="g_t4")
            nc.scalar.activation(gt[:], ctab[:], AF.Identity, scale=a_im[:])
            nc.scalar.activation(g_t4[:], stab[:], AF.Identity, scale=a_re[:])
            nc.vector.tensor_tensor(gt[:], gt[:], g_t4[:], OP.subtract)
            nc.vector.tensor_tensor(mdt(g_r[:]), gdr[:], gn2[:], OP.mult)
            nc.vector.tensor_tensor(mdt(g_i[:]), gt[:], gn2[:], OP.mult)

            # hi halves to base-partition-0 tiles (matmul rhs must match lhsT base)
            g_r_hi = gpool.tile([64, 4096], f32, tag="g_r_hi")
            g_i_hi = gpool.tile([64, 4096], f32, tag="g_i_hi")
            nc.sync.dma_start(mdt(g_r_hi[:]), mdt(g_r[64:128, :]))
            nc.sync.dma_start(mdt(g_i_hi[:]), mdt(g_i[64:128, :]))

            # ---------- Ks rows = BC @ G -> DRAM scratch ----------
            for j in range(16):  # m chunks of 512
                half = j // 8
                foff = (j % 8) * 512
                gr_sl = (g_r if half == 0 else g_r_hi)[0:64, foff:foff + 512]
                gi_sl = (g_i if half == 0 else g_i_hi)[0:64, foff:foff + 512]
                kr = pskp.tile([HSH, 512], f32, tag="ks_ps")
                ki = pskp.tile([HSH, 512], f32, tag="ks_ps")
                nc.tensor.matmul(kr[:], mdt(bct_r[:]), mdt(gr_sl), start=True, stop=False)
                nc.tensor.matmul(kr[:], mdt(bct_in[:]), mdt(gi_sl), start=False, stop=True)
                nc.tensor.matmul(ki[:], mdt(bct_i[:]), mdt(gr_sl), start=True, stop=False)
                nc.tensor.matmul(ki[:], mdt(bct_r[:]), mdt(gi_sl), start=False, stop=True)
                krs = ksp.tile([HSH, 512], f32, tag="ks_sb")
                kis = ksp.tile([HSH, 512], f32, tag="ks_sb")
                nc.scalar.activation(krs[:], kr[:], AF.Copy)
                nc.scalar.activation(kis[:], ki[:], AF.Copy)
                nc.sync.dma_start(ksr_d[:, j * 512:(j + 1) * 512], krs[:])
                nc.sync.dma_start(ksi_d[:, j * 512:(j + 1) * 512], kis[:])

            prologue_pools[1].__exit__(None, None, None)
            prologue_pools[0].__exit__(None, None, None)
            main_pools = (
                tc.tile_pool(name="io", bufs=IOBUFS),
                tc.tile_pool(name="mid", bufs=MIDBUFS),
                tc.tile_pool(name="ps", bufs=2, space=bass.MemorySpace.PSUM),
            )
            iop = main_pools[0].__enter__()
            midp = main_pools[1].__enter__()
            psp = main_pools[2].__enter__()
            # ---------- main loop: two interleaved h-lanes ----------
            SC = 1.0 / L

            def do_group(h, g, lane, kb):
                ksr_b, ksi_b, tr_b, ti_b, tir_b, tii_b = kb
                sfx = str(lane)
                bsl = slice(g * GB, (g + 1) * GB)
                u_t = iop.tile([64, GB, 128], f32, tag="u_t" + sfx, name="u_t")
                nc.sync.dma_start(
                    mdt(u_t[:]),
                    mdt(u_d[bsl, h, :].rearrange("b (n2 n1) -> n2 b n1", n1=128)))
                u_flat = u_t[:].rearrange("p b f -> p (b f)")

                # fwd stage 1
                y1r = psp.tile([64, 512], f32, tag="psA" + sfx, name="y1r")
                y1i = psp.tile([64, 512], f32, tag="psA" + sfx, name="y1i")
                nc.tensor.matmul(y1r[:], mdt(tb["f64r"][:]), mdt(u_flat))
                nc.tensor.matmul(y1i[:], mdt(tb["f64i"][:]), mdt(u_flat))

                # twiddle
                y1r_s = midp.tile([64, GB, 128], f32, tag="y1r_s" + sfx, name="y1r_s")
                y1i_s = midp.tile([64, GB, 128], f32, tag="y1i_s" + sfx, name="y1i_s")
                nc.scalar.activation(y1r_s[:].rearrange("p b f -> p (b f)"), y1r[:], AF.Copy)
                nc.scalar.activation(y1i_s[:].rearrange("p b f -> p (b f)"), y1i[:], AF.Copy)
                y2r = midp.tile([64, GB, 128], f32, tag="y2r" + sfx, name="y2r")
                y2i = midp.tile([64, GB, 128], f32, tag="y2i" + sfx, name="y2i")
                tw1 = midp.tile([64, GB, 128], f32, tag="tw1" + sfx, name="tw1")
                tw2 = midp.tile([64, GB, 128], f32, tag="tw2" + sfx, name="tw2")
                nc.vector.tensor_tensor(y2r[:], y1r_s[:], tr_b, OP.mult)
                nc.gpsimd.tensor_tensor(tw1[:], y1i_s[:], ti_b, OP.mult)
                nc.vector.tensor_tensor(y2r[:], y2r[:], tw1[:], OP.subtract)
                nc.vector.tensor_tensor(y2i[:], y1r_s[:], ti_b, OP.mult)
                nc.vector.tensor_tensor(tw2[:], y1i_s[:], tr_b, OP.mult)
                nc.gpsimd.tensor_tensor(y2i[:], y2i[:], tw2[:], OP.add)

                # fwd transposes
                y2t_ps = psp.tile([128, 512], f32, tag="psB" + sfx, name="y2t_ps")
                for j2 in range(GB):
                    nc.tensor.transpose(y2t_ps[:, j2 * 64:(j2 + 1) * 64],
                                        y2r[:, j2, :], tb["i64"][:])
                    nc.tensor.transpose(y2t_ps[:, 256 + j2 * 64:256 + (j2 + 1) * 64],
                                        y2i[:, j2, :], tb["i64"][:])
                y2t = midp.tile([128, 512], f32, tag="y2t" + sfx, name="y2t")
                nc.scalar.activation(mdt(y2t[:]), y2t_ps[:], AF.Copy)
                y2tr, y2ti = y2t[:, 0:256], y2t[:, 256:512]

                # fwd stage 2
                x_ps = psp.tile([128, 512], f32, tag="psB" + sfx, name="x_ps")
                xr, xi = x_ps[:, 0:256], x_ps[:, 256:512]
                nc.tensor.matmul(xr, mdt(tb["f128r"][:]), mdt(y2tr), start=True, stop=False)
                nc.tensor.matmul(xr, mdt(tb["f128in"][:]), mdt(y2ti), start=False, stop=True)
                nc.tensor.matmul(xi, mdt(tb["f128i"][:]), mdt(y2tr), start=True, stop=False)
                nc.tensor.matmul(xi, mdt(tb["f128r"][:]), mdt(y2ti), start=False, stop=True)

                # spectral
                xr_s = midp.tile([128, GB, 64], f32, tag="xr_s" + sfx, name="xr_s")
                xi_s = midp.tile([128, GB, 64], f32, tag="xi_s" + sfx, name="xi_s")
                nc.scalar.activation(xr_s[:].rearrange("p b f -> p (b f)"), xr, AF.Copy)
                nc.scalar.activation(xi_s[:].rearrange("p b f -> p (b f)"), xi, AF.Copy)
                s_sb = midp.tile([128, 2, GB, 64], f32, tag="s_sb" + sfx, name="s_sb")
                sr, si = s_sb[:, 0], s_sb[:, 1]
                sw1 = midp.tile([128, GB, 64], f32, tag="sw1" + sfx, name="sw1")
                sw2 = midp.tile([128, GB, 64], f32, tag="sw2" + sfx, name="sw2")
                nc.vector.tensor_tensor(mdt(sr[:]), xr_s[:], ksr_b, OP.mult)
                nc.gpsimd.tensor_tensor(sw1[:], xi_s[:], ksi_b, OP.mult)
                nc.vector.tensor_tensor(mdt(sr[:]), sr[:], sw1[:], OP.subtract)
                nc.vector.tensor_tensor(mdt(si[:]), xr_s[:], ksi_b, OP.mult)
                nc.vector.tensor_tensor(sw2[:], xi_s[:], ksr_b, OP.mult)
                nc.gpsimd.tensor_tensor(mdt(si[:]), si[:], sw2[:], OP.add)
                sr_f = sr.rearrange("p b f -> p (b f)")
                si_f = si.rearrange("p b f -> p (b f)")

                # inv stage 1
                z1_ps = psp.tile([128, 512], f32, tag="psB" + sfx, name="z1_ps")
                z1r, z1i = z1_ps[:, 0:256], z1_ps[:, 256:512]
                nc.tensor.matmul(z1r, mdt(tb["f128r"][:]), mdt(sr_f), start=True, stop=False)
                nc.tensor.matmul(z1r, mdt(tb["f128i"][:]), mdt(si_f), start=False, stop=True)
                nc.tensor.matmul(z1i, mdt(tb["f128r"][:]), mdt(si_f), start=True, stop=False)
                nc.tensor.matmul(z1i, mdt(tb["f128in"][:]), mdt(sr_f), start=False, stop=True)

                # inv twiddle
                z1r_s = midp.tile([128, GB, 64], f32, tag="z1r_s" + sfx, name="z1r_s")
                z1i_s = midp.tile([128, GB, 64], f32, tag="z1i_s" + sfx, name="z1i_s")
                nc.scalar.activation(z1r_s[:].rearrange("p b f -> p (b f)"), z1r, AF.Copy)
                nc.scalar.activation(z1i_s[:].rearrange("p b f -> p (b f)"), z1i, AF.Copy)
                z2r = midp.tile([128, GB, 64], f32, tag="z2r" + sfx, name="z2r")
                z2i = midp.tile([128, GB, 64], f32, tag="z2i" + sfx, name="z2i")
                zw1 = midp.tile([128, GB, 64], f32, tag="zw1" + sfx, name="zw1")
                zw2 = midp.tile([128, GB, 64], f32, tag="zw2" + sfx, name="zw2")
                nc.vector.tensor_tensor(z2r[:], z1r_s[:], tir_b, OP.mult)
                nc.gpsimd.tensor_tensor(zw1[:], z1i_s[:], tii_b, OP.mult)
                nc.vector.tensor_tensor(z2r[:], z2r[:], zw1[:], OP.subtract)
                nc.vector.tensor_tensor(z2i[:], z1r_s[:], tii_b, OP.mult)
                nc.vector.tensor_tensor(zw2[:], z1i_s[:], tir_b, OP.mult)
                nc.gpsimd.tensor_tensor(z2i[:], z2i[:], zw2[:], OP.add)

                # inv transposes
                z2tr_ps = psp.tile([64, 512], f32, tag="psA" + sfx, name="z2tr_ps")
                z2ti_ps = psp.tile([64, 512], f32, tag="psA" + sfx, name="z2ti_ps")
                for j2 in range(GB):
                    nc.tensor.transpose(z2tr_ps[:, j2 * 128:(j2 + 1) * 128],
                                        z2r[:, j2, :], tb["i128"][:])
                    nc.tensor.transpose(z2ti_ps[:, j2 * 128:(j2 + 1) * 128],
                                        z2i[:, j2, :], tb["i128"][:])
                z2t = midp.tile([64, 1024], f32, tag="z2t" + sfx, name="z2t")
                nc.scalar.activation(mdt(z2t[:, 0:512]), z2tr_ps[:], AF.Copy)
                nc.scalar.activation(mdt(z2t[:, 512:1024]), z2ti_ps[:], AF.Copy)

                # inv stage 2 (real part)
                xo_ps = psp.tile([64, 512], f32, tag="psB" + sfx, name="xo_ps")
                nc.tensor.matmul(xo_ps[:], mdt(tb["f64r"][:]), mdt(z2t[:, 0:512]),
                                 start=True, stop=False)
                nc.tensor.matmul(xo_ps[:], mdt(tb["f64i"][:]), mdt(z2t[:, 512:1024]),
                                 start=False, stop=True)

                # final
                ud = midp.tile([64, 512], f32, tag="ud" + sfx, name="ud")
                nc.gpsimd.tensor_scalar_mul(ud[:], u_flat, d_b[:, h:h + 1])
                yt = midp.tile([64, 512], f32, tag="yt" + sfx, name="yt")
                nc.vector.scalar_tensor_tensor(yt[:], xo_ps[:], SC, ud[:],
                                               OP.mult, OP.add)
                yo = iop.tile([64, GB, 128], f32, tag="yo" + sfx, name="yo")
                nc.scalar.activation(yo[:].rearrange("p b f -> p (b f)"), yt[:], AF.Tanh)
                nc.sync.dma_start(
                    y_d[bsl, h, :].rearrange("b (n2 n1) -> n2 b n1", n1=128), yo[:])

            def prep_h(h, lane):
                sfx = str(lane)
                ksr_t = ksp.tile([128, 64], f32, tag="ks_h" + sfx, name="ksr_t")
                ksi_t = ksp.tile([128, 64], f32, tag="ks_h" + sfx, name="ksi_t")
                nc.sync.dma_start(ksr_t[:], ksr_d[h].rearrange("(k1 k2) -> k1 k2", k2=64))
                nc.sync.dma_start(ksi_t[:], ksi_d[h].rearrange("(k1 k2) -> k1 k2", k2=64))
                return (
                    ksr_t[:].unsqueeze(1).broadcast_to([128, GB, 64]),
                    ksi_t[:].unsqueeze(1).broadcast_to([128, GB, 64]),
                    tb["tr"][:].unsqueeze(1).broadcast_to([64, GB, 128]),
                    tb["ti"][:].unsqueeze(1).broadcast_to([64, GB, 128]),
                    tb["tir"][:].unsqueeze(1).broadcast_to([128, GB, 64]),
                    tb["tii"][:].unsqueeze(1).broadcast_to([128, GB, 64]),
                )

            for _rep in range(REPEAT):
                for hp in range(HSH // 2):
                    hA, hB = 2 * hp, 2 * hp + 1
                    kbA = prep_h(hA, 0)
                    kbB = prep_h(hB, 1)
                    for g in range(NG):
                        do_group(hA, g, 0, kbA)
                        do_group(hB, g, 1, kbB)
            for mp in reversed(main_pools):
                mp.__exit__(None, None, None)

    nc.compile()
    return nc


def _get_program():
    key = ("prog", F32R, REPEAT, MIDBUFS, IOBUFS)
    if key not in _CACHE:
        import concourse.bass as bass
        import concourse.tile as tile
        from concourse import mybir, bacc
        _CACHE[key] = _build((bass, tile, mybir, bacc))
    return _CACHE[key]


def kernel(u, A_re, A_im, BC_re, BC_im, D):
    from concourse.bass_utils import run_bass_kernel_spmd

    u = np.ascontiguousarray(u, dtype=np.float32)
    tabs = _tables()
    nc = _get_program()

    in_maps = []
    for c in range(NCORES):
        hs = slice(c * HSH, (c + 1) * HSH)
        m = {
            "u_sh": np.ascontiguousarray(u[:, hs, :]),
            "a_re": np.ascontiguousarray(
                np.concatenate([A_re, A_re]).reshape(2 * P, 1).astype(np.float32)),
            "a_im": np.ascontiguousarray(
                np.concatenate([A_im, A_im]).reshape(2 * P, 1).astype(np.float32)),
            "bct_r": np.ascontiguousarray(BC_re[hs].T.astype(np.float32)),
            "bct_i": np.ascontiguousarray(BC_im[hs].T.astype(np.float32)),
            "bct_i_neg": np.ascontiguousarray(-BC_im[hs].T.astype(np.float32)),
            "d_sh": np.ascontiguousarray(D[hs].reshape(1, HSH).astype(np.float32)),
        }
        m.update(tabs)
        in_maps.append(m)

    res = None
    last_err = None
    for attempt in range(3):
        try:
            res = run_bass_kernel_spmd(nc, in_maps, list(range(NCORES)))
            break
        except Exception as e:  # transient NRT_EXEC_UNIT_UNRECOVERABLE flakes
            last_err = e
            import time as _time
            _time.sleep(2.0)
    if res is None:
        raise last_err
    out = np.concatenate([res.results[c]["y_sh"] for c in range(NCORES)], axis=1)
    return out.astype(np.float32)


if __name__ == "__main__":
    rng = np.random.default_rng(0)
    u = rng.standard_normal((B, H, L), dtype=np.float32)
    A_re = rng.uniform(0.5, 0.99, P).astype(np.float32)
    A_im = rng.uniform(-0.5, 0.5, P).astype(np.float32)
    BC_re = rng.standard_normal((H, P), dtype=np.float32)
    BC_im = rng.standard_normal((H, P), dtype=np.float32)
    D = rng.uniform(0, 1, H).astype(np.float32)
    y = kernel(u=u, A_re=A_re, A_im=A_im, BC_re=BC_re, BC_im=BC_im, D=D)
    print("out", y.shape, y.dtype)



# revision 1
# speedup vs baseline: 1.1026x; 1.1026x over previous
"""Trainium2 Bass kernel for nn_FFTConv: y = tanh(Re(ifft(fft(u)*Ks)) + D*u).

Self-contained: builds constant tables with numpy, shards over 8 NeuronCores
(H-parallel: 32 channels/core), runs a Bass/Tile kernel per core via
run_bass_kernel_spmd, gathers the full output.

Algorithm (per core):
  Prologue:
    G[p,m] = 1/(1 - A_p * WL^m)           (P=64 poles x L=8192 freqs, on DVE/ACT)
    Ks[h]  = BC[h,:] @ G                  (TensorE, -> DRAM scratch, complex)
  Main loop over (h, b-group of 4):  2-stage matmul FFT, L = 128*64
    n = n1 + 128*n2 ; m = k2 + 64*k1
    Y1 = F64 @ u.reshape(64,128)          [k2, n1]
    Y2 = Y1 * T                           twiddle T[k2,n1] = WL^(n1*k2)
    X  = F128 @ Y2.T                      [k1, k2]  (PE transpose between)
    S  = X * Ks[h].reshape(128,64)
    Z1 = conj(F128) @ S                   [o2, k2]
    Z2 = Z1 * conj(TI)                    TI[o2,k2] = WL^(k2*o2)
    xo = Re(conj(F64) @ Z2.T)             [o1, o2]  (PE transpose between)
    y  = tanh(xo/L + D[h]*u)
"""
import os
import sys
import numpy as np

for p in ("/opt/trn_rl_repo", "/root/.axon_site/_ro/trn_rl_repo"):
    if os.path.isdir(p) and p not in sys.path:
        sys.path.append(p)

B, H, L, P = 16, 256, 8192, 64
NCORES = 8
HSH = H // NCORES          # 32 channels per core
GB = 4                     # b-group size (pairs per inner group)
NG = B // GB               # 4 groups per h
F32R = os.environ.get("KERNEL_F32R", "0") == "1"   # reduced-precision fast matmul mode
REPEAT = int(os.environ.get("KERNEL_REPEAT", "1"))  # repeat main loop (timing only)
MIDBUFS = int(os.environ.get("KERNEL_MIDBUFS", "2"))
IOBUFS = int(os.environ.get("KERNEL_IOBUFS", "3"))

_CACHE = {}


def _tables():
    a64 = np.arange(64)
    a128 = np.arange(128)
    th64 = 2 * np.pi * np.outer(a64, a64) / 64.0
    th128 = 2 * np.pi * np.outer(a128, a128) / 128.0
    thT = 2 * np.pi * np.outer(a64, a128) / L       # [k2, n1]
    thTI = 2 * np.pi * np.outer(a128, a64) / L      # [o2, k2]
    t = {
        "f64r": np.cos(th64), "f64i": -np.sin(th64),
        "f128r": np.cos(th128), "f128i": -np.sin(th128), "f128in": np.sin(th128),
        "tr": np.cos(thT), "ti": -np.sin(thT),
        # conj(TI) passed directly: re=cos, im=+sin
        "tir": np.cos(thTI), "tii": np.sin(thTI),
        "i64": np.eye(64), "i128": np.eye(128),
    }
    m = np.arange(L)
    cm = np.cos(2 * np.pi * m / L).reshape(2, 4096)
    sm = np.sin(2 * np.pi * m / L).reshape(2, 4096)
    # pre-replicated across 64 partitions per half: (128, 4096)
    t["cm"] = np.repeat(cm, 64, axis=0)
    t["sm"] = np.repeat(sm, 64, axis=0)
    return {k: v.astype(np.float32) for k, v in t.items()}


def _build(nc_mod):
    """Builds the Bass program (same program for all cores)."""
    bass, tile, mybir, bacc = nc_mod
    dt = mybir.dt
    f32 = dt.float32
    MMDT = dt.float32r if F32R else dt.float32

    def mdt(ap):
        return ap.bitcast(MMDT) if F32R else ap

    nc = bacc.Bacc("TRN2", target_bir_lowering=False, debug=False)
    AF = mybir.ActivationFunctionType
    OP = mybir.AluOpType

    # ---------------- DRAM parameters ----------------
    u_d = nc.declare_dram_parameter("u_sh", [B, HSH, L], f32, isOutput=False)
    y_d = nc.declare_dram_parameter("y_sh", [B, HSH, L], f32, isOutput=True)
    ar_d = nc.declare_dram_parameter("a_re", [2 * P, 1], f32, isOutput=False)
    ai_d = nc.declare_dram_parameter("a_im", [2 * P, 1], f32, isOutput=False)
    bcr_d = nc.declare_dram_parameter("bct_r", [P, HSH], f32, isOutput=False)
    bci_d = nc.declare_dram_parameter("bct_i", [P, HSH], f32, isOutput=False)
    bcin_d = nc.declare_dram_parameter("bct_i_neg", [P, HSH], f32, isOutput=False)
    d_d = nc.declare_dram_parameter("d_sh", [1, HSH], f32, isOutput=False)
    tbl_names = ["f64r", "f64i", "f128r", "f128i", "f128in",
                 "tr", "ti", "tir", "tii", "i64", "i128", "cm", "sm"]
    tbl_shapes = {"f64r": [64, 64], "f64i": [64, 64],
                  "f128r": [128, 128], "f128i": [128, 128], "f128in": [128, 128],
                  "tr": [64, 128], "ti": [64, 128],
                  "tir": [128, 64], "tii": [128, 64],
                  "i64": [64, 64], "i128": [128, 128],
                  "cm": [128, 4096], "sm": [128, 4096]}
    tbl_d = {n: nc.declare_dram_parameter(n, tbl_shapes[n], f32, isOutput=False) for n in tbl_names}

    ksr_d = nc.dram_tensor("ksr_scratch", [HSH, L], f32)
    ksi_d = nc.dram_tensor("ksi_scratch", [HSH, L], f32)

    with tile.TileContext(nc) as tc:
        with (
            tc.tile_pool(name="const", bufs=1) as cpool,
            tc.tile_pool(name="ks", bufs=2) as ksp,
        ):
            # ---------- load constants ----------
            tb = {}
            for n in tbl_names:
                if n in ("cm", "sm"):
                    continue
                tb[n] = cpool.tile(tbl_shapes[n], f32, tag=f"c_{n}", name=f"c_{n}")
                if n in ("f64r", "f64i", "f128r", "f128i", "f128in"):
                    nc.sync.dma_start(mdt(tb[n][:]), mdt(tbl_d[n][:]))
                else:
                    nc.sync.dma_start(tb[n][:], tbl_d[n][:])
            a_re = cpool.tile([128, 1], f32, tag="a_re")
            a_im = cpool.tile([128, 1], f32, tag="a_im")
            nc.sync.dma_start(a_re[:], ar_d[:])
            nc.sync.dma_start(a_im[:], ai_d[:])
            bct_r = cpool.tile([P, HSH], f32, tag="bct_r")
            bct_i = cpool.tile([P, HSH], f32, tag="bct_i")
            bct_in = cpool.tile([P, HSH], f32, tag="bct_in")
            nc.sync.dma_start(mdt(bct_r[:]), mdt(bcr_d[:]))
            nc.sync.dma_start(mdt(bct_i[:]), mdt(bci_d[:]))
            nc.sync.dma_start(mdt(bct_in[:]), mdt(bcin_d[:]))
            # D broadcast to 64 partitions
            d_b = cpool.tile([64, HSH], f32, tag="d_b")
            nc.sync.dma_start(d_b[:], d_d[:].broadcast_to([64, HSH]))

            prologue_pools = (
                tc.tile_pool(name="gwork", bufs=1),
                tc.tile_pool(name="psk", bufs=2, space=bass.MemorySpace.PSUM),
            )
            gpool = prologue_pools[0].__enter__()
            pskp = prologue_pools[1].__enter__()
            # 1 + |A|^2 per partition (stacked twice)
            one_a2 = cpool.tile([128, 1], f32, tag="one_a2")
            t_sq = gpool.tile([128, 1], f32, tag="g_sq")
            nc.scalar.activation(one_a2[:], a_re[:], AF.Square)
            nc.scalar.activation(t_sq[:], a_im[:], AF.Square)
            nc.vector.tensor_tensor(one_a2[:], one_a2[:], t_sq[:], OP.add)
            nc.vector.tensor_scalar_add(one_a2[:], one_a2[:], 1.0)

            # ---------- G = 1/(1 - A*WL^m), layout (128p=[p|p], 4096f) ----------
            ctab = gpool.tile([128, 4096], f32, tag="g_ctab")
            stab = gpool.tile([128, 4096], f32, tag="g_stab")
            nc.sync.dma_start(ctab[:], tbl_d["cm"][:])
            nc.sync.dma_start(stab[:], tbl_d["sm"][:])
            gq = gpool.tile([128, 4096], f32, tag="g_q")
            gt = gpool.tile([128, 4096], f32, tag="g_t")
            gdr = gpool.tile([128, 4096], f32, tag="g_dr")
            gn2 = gpool.tile([128, 4096], f32, tag="g_n2")
            g_r = gpool.tile([128, 4096], f32, tag="g_r")
            g_i = gpool.tile([128, 4096], f32, tag="g_i")
            # q = A_re*C + A_im*S
            nc.vector.tensor_scalar_mul(gq[:], ctab[:], a_re[:])
            nc.scalar.activation(gt[:], stab[:], AF.Identity, scale=a_im[:])
            nc.vector.tensor_tensor(gq[:], gq[:], gt[:], OP.add)
            # dr = 1 - q ; n2 = 1+|A|^2 - 2q ; rn = 1/n2
            nc.scalar.activation(gdr[:], gq[:], AF.Identity, scale=-1.0, bias=1.0)
            nc.vector.tensor_scalar(gn2[:], gq[:], -2.0, one_a2[:], OP.mult, OP.add)
            nc.vector.reciprocal(gn2[:], gn2[:])
            # di_n = A_im*C - A_re*S  (numerator of +Gi)
            g_t4 = gpool.tile([128, 4096], f32, tag="g_t4")
            nc.scalar.activation(gt[:], ctab[:], AF.Identity, scale=a_im[:])
            nc.scalar.activation(g_t4[:], stab[:], AF.Identity, scale=a_re[:])
            nc.vector.tensor_tensor(gt[:], gt[:], g_t4[:], OP.subtract)
            nc.vector.tensor_tensor(mdt(g_r[:]), gdr[:], gn2[:], OP.mult)
            nc.vector.tensor_tensor(mdt(g_i[:]), gt[:], gn2[:], OP.mult)

            # hi halves to base-partition-0 tiles (matmul rhs must match lhsT base)
            g_r_hi = gpool.tile([64, 4096], f32, tag="g_r_hi")
            g_i_hi = gpool.tile([64, 4096], f32, tag="g_i_hi")
            nc.sync.dma_start(mdt(g_r_hi[:]), mdt(g_r[64:128, :]))
            nc.sync.dma_start(mdt(g_i_hi[:]), mdt(g_i[64:128, :]))

            # ---------- Ks rows = BC @ G -> DRAM scratch ----------
            for j in range(16):  # m chunks of 512
                half = j // 8
                foff = (j % 8) * 512
                gr_sl = (g_r if half == 0 else g_r_hi)[0:64, foff:foff + 512]
                gi_sl = (g_i if half == 0 else g_i_hi)[0:64, foff:foff + 512]
                kr = pskp.tile([HSH, 512], f32, tag="ks_ps")
                ki = pskp.tile([HSH, 512], f32, tag="ks_ps")
                nc.tensor.matmul(kr[:], mdt(bct_r[:]), mdt(gr_sl), start=True, stop=False)
                nc.tensor.matmul(kr[:], mdt(bct_in[:]), mdt(gi_sl), start=False, stop=True)
                nc.tensor.matmul(ki[:], mdt(bct_i[:]), mdt(gr_sl), start=True, stop=False)
                nc.tensor.matmul(ki[:], mdt(bct_r[:]), mdt(gi_sl), start=False, stop=True)
                krs = ksp.tile([HSH, 512], f32, tag="ks_sb")
                kis = ksp.tile([HSH, 512], f32, tag="ks_sb")
                nc.scalar.activation(krs[:], kr[:], AF.Copy)
                nc.scalar.activation(kis[:], ki[:], AF.Copy)
                nc.sync.dma_start(ksr_d[:, j * 512:(j + 1) * 512], krs[:])
                nc.sync.dma_start(ksi_d[:, j * 512:(j + 1) * 512], kis[:])

            prologue_pools[1].__exit__(None, None, None)
            prologue_pools[0].__exit__(None, None, None)
            main_pools = (
                tc.tile_pool(name="io", bufs=IOBUFS),
                tc.tile_pool(name="mid", bufs=MIDBUFS),
                tc.tile_pool(name="ps", bufs=2, space=bass.MemorySpace.PSUM),
            )
            iop = main_pools[0].__enter__()
            midp = main_pools[1].__enter__()
            psp = main_pools[2].__enter__()
            # ---------- main loop: two interleaved h-lanes ----------
            SC = 1.0 / L

            def do_group(h, g, lane, kb):
                ksr_b, ksi_b, tr_b, ti_b, tir_b, tii_b = kb
                sfx = str(lane)
                bsl = slice(g * GB, (g + 1) * GB)
                u_t = iop.tile([64, GB, 128], f32, tag="u_t" + sfx, name="u_t")
                nc.sync.dma_start(
                    mdt(u_t[:]),
                    mdt(u_d[bsl, h, :].rearrange("b (n2 n1) -> n2 b n1", n1=128)))
                u_flat = u_t[:].rearrange("p b f -> p (b f)")

                # fwd stage 1
                y1r = psp.tile([64, 512], f32, tag="psA" + sfx, name="y1r")
                y1i = psp.tile([64, 512], f32, tag="psA" + sfx, name="y1i")
                nc.tensor.matmul(y1r[:], mdt(tb["f64r"][:]), mdt(u_flat))
                nc.tensor.matmul(y1i[:], mdt(tb["f64i"][:]), mdt(u_flat))

                # twiddle
                y1r_s = midp.tile([64, GB, 128], f32, tag="y1r_s" + sfx, name="y1r_s")
                y1i_s = midp.tile([64, GB, 128], f32, tag="y1i_s" + sfx, name="y1i_s")
                nc.scalar.activation(y1r_s[:].rearrange("p b f -> p (b f)"), y1r[:], AF.Copy)
                nc.scalar.activation(y1i_s[:].rearrange("p b f -> p (b f)"), y1i[:], AF.Copy)
                y2r = midp.tile([64, GB, 128], f32, tag="y2r" + sfx, name="y2r")
                y2i = midp.tile([64, GB, 128], f32, tag="y2i" + sfx, name="y2i")
                tw1 = midp.tile([64, GB, 128], f32, tag="tw1" + sfx, name="tw1")
                tw2 = midp.tile([64, GB, 128], f32, tag="tw2" + sfx, name="tw2")
                nc.vector.tensor_tensor(y2r[:], y1r_s[:], tr_b, OP.mult)
                nc.gpsimd.tensor_tensor(tw1[:], y1i_s[:], ti_b, OP.mult)
                nc.vector.tensor_tensor(y2r[:], y2r[:], tw1[:], OP.subtract)
                nc.vector.tensor_tensor(y2i[:], y1r_s[:], ti_b, OP.mult)
                nc.vector.tensor_tensor(tw2[:], y1i_s[:], tr_b, OP.mult)
                nc.gpsimd.tensor_tensor(y2i[:], y2i[:], tw2[:], OP.add)

                # fwd transposes
                y2t_ps = psp.tile([128, 512], f32, tag="psB" + sfx, name="y2t_ps")
                for j2 in range(GB):
                    nc.tensor.transpose(y2t_ps[:, j2 * 64:(j2 + 1) * 64],
                                        y2r[:, j2, :], tb["i64"][:])
                    nc.tensor.transpose(y2t_ps[:, 256 + j2 * 64:256 + (j2 + 1) * 64],
                                        y2i[:, j2, :], tb["i64"][:])
                y2t = midp.tile([128, 512], f32, tag="y2t" + sfx, name="y2t")
                nc.scalar.activation(mdt(y2t[:]), y2t_ps[:], AF.Copy)
                y2tr, y2ti = y2t[:, 0:256], y2t[:, 256:512]

                # fwd stage 2
                x_ps = psp.tile([128, 512], f32, tag="psB" + sfx, name="x_ps")
                xr, xi = x_ps[:, 0:256], x_ps[:, 256:512]
                nc.tensor.matmul(xr, mdt(tb["f128r"][:]), mdt(y2tr), start=True, stop=False)
                nc.tensor.matmul(xr, mdt(tb["f128in"][:]), mdt(y2ti), start=False, stop=True)
                nc.tensor.matmul(xi, mdt(tb["f128i"][:]), mdt(y2tr), start=True, stop=False)
                nc.tensor.matmul(xi, mdt(tb["f128r"][:]), mdt(y2ti), start=False, stop=True)

                # spectral
                xr_s = midp.tile([128, GB, 64], f32, tag="xr_s" + sfx, name="xr_s")
                xi_s = midp.tile([128, GB, 64], f32, tag="xi_s" + sfx, name="xi_s")
                nc.scalar.activation(xr_s[:].rearrange("p b f -> p (b f)"), xr, AF.Copy)
                nc.scalar.activation(xi_s[:].rearrange("p b f -> p (b f)"), xi, AF.Copy)
                s_sb = midp.tile([128, 2, GB, 64], f32, tag="s_sb" + sfx, name="s_sb")
                sr, si = s_sb[:, 0], s_sb[:, 1]
                sw1 = midp.tile([128, GB, 64], f32, tag="sw1" + sfx, name="sw1")
                sw2 = midp.tile([128, GB, 64], f32, tag="sw2" + sfx, name="sw2")
                nc.vector.tensor_tensor(mdt(sr[:]), xr_s[:], ksr_b, OP.mult)
                nc.gpsimd.tensor_tensor(sw1[:], xi_s[:], ksi_b, OP.mult)
                nc.vector.tensor_tensor(mdt(sr[:]), sr[:], sw1[:], OP.subtract)
                nc.vector.tensor_tensor(mdt(si[:]), xr_s[:], ksi_b, OP.mult)
                nc.vector.tensor_tensor(sw2[:], xi_s[:], ksr_b, OP.mult)
                nc.gpsimd.tensor_tensor(mdt(si[:]), si[:], sw2[:], OP.add)
                sr_f = sr.rearrange("p b f -> p (b f)")
                si_f = si.rearrange("p b f -> p (b f)")

                # inv stage 1
                z1_ps = psp.tile([128, 512], f32, tag="psB" + sfx, name="z1_ps")
                z1r, z1i = z1_ps[:, 0:256], z1_ps[:, 256:512]
                nc.tensor.matmul(z1r, mdt(tb["f128r"][:]), mdt(sr_f), start=True, stop=False)
                nc.tensor.matmul(z1r, mdt(tb["f128i"][:]), mdt(si_f), start=False, stop=True)
                nc.tensor.matmul(z1i, mdt(tb["f128r"][:]), mdt(si_f), start=True, stop=False)
                nc.tensor.matmul(z1i, mdt(tb["f128in"][:]), mdt(sr_f), start=False, stop=True)

                # inv twiddle
                z1r_s = midp.tile([128, GB, 64], f32, tag="z1r_s" + sfx, name="z1r_s")
                z1i_s = midp.tile([128, GB, 64], f32, tag="z1i_s" + sfx, name="z1i_s")
                nc.scalar.activation(z1r_s[:].rearrange("p b f -> p (b f)"), z1r, AF.Copy)
                nc.scalar.activation(z1i_s[:].rearrange("p b f -> p (b f)"), z1i, AF.Copy)
                z2r = midp.tile([128, GB, 64], f32, tag="z2r" + sfx, name="z2r")
                z2i = midp.tile([128, GB, 64], f32, tag="z2i" + sfx, name="z2i")
                zw1 = midp.tile([128, GB, 64], f32, tag="zw1" + sfx, name="zw1")
                zw2 = midp.tile([128, GB, 64], f32, tag="zw2" + sfx, name="zw2")
                nc.vector.tensor_tensor(z2r[:], z1r_s[:], tir_b, OP.mult)
                nc.gpsimd.tensor_tensor(zw1[:], z1i_s[:], tii_b, OP.mult)
                nc.vector.tensor_tensor(z2r[:], z2r[:], zw1[:], OP.subtract)
                nc.vector.tensor_tensor(z2i[:], z1r_s[:], tii_b, OP.mult)
                nc.vector.tensor_tensor(zw2[:], z1i_s[:], tir_b, OP.mult)
                nc.gpsimd.tensor_tensor(z2i[:], z2i[:], zw2[:], OP.add)

                # inv transposes
                z2tr_ps = psp.tile([64, 512], f32, tag="psA" + sfx, name="z2tr_ps")
                z2ti_ps = psp.tile([64, 512], f32, tag="psA" + sfx, name="z2ti_ps")
                for j2 in range(GB):
                    nc.tensor.transpose(z2tr_ps[:, j2 * 128:(j2 + 1) * 128],
                                        z2r[:, j2, :], tb["i128"][:])
                    nc.tensor.transpose(z2ti_ps[:, j2 * 128:(j2 + 1) * 128],
                                        z2i[:, j2, :], tb["i128"][:])
                z2t = midp.tile([64, 1024], f32, tag="z2t" + sfx, name="z2t")
                nc.scalar.activation(mdt(z2t[:, 0:512]), z2tr_ps[:], AF.Copy)
                nc.scalar.activation(mdt(z2t[:, 512:1024]), z2ti_ps[:], AF.Copy)

                # inv stage 2 (real part)
                xo_ps = psp.tile([64, 512], f32, tag="psB" + sfx, name="xo_ps")
                nc.tensor.matmul(xo_ps[:], mdt(tb["f64r"][:]), mdt(z2t[:, 0:512]),
                                 start=True, stop=False)
                nc.tensor.matmul(xo_ps[:], mdt(tb["f64i"][:]), mdt(z2t[:, 512:1024]),
                                 start=False, stop=True)

                # final
                ud = midp.tile([64, 512], f32, tag="ud" + sfx, name="ud")
                nc.gpsimd.tensor_scalar_mul(ud[:], u_flat, d_b[:, h:h + 1])
                yt = midp.tile([64, 512], f32, tag="yt" + sfx, name="yt")
                nc.vector.scalar_tensor_tensor(yt[:], xo_ps[:], SC, ud[:],
                                               OP.mult, OP.add)
                yo = iop.tile([64, GB, 128], f32, tag="yo" + sfx, name="yo")
                nc.scalar.activation(yo[:].rearrange("p b f -> p (b f)"), yt[:], AF.Tanh)
                nc.sync.dma_start(
                    y_d[bsl, h, :].rearrange("b (n2 n1) -> n2 b n1", n1=128), yo[:])

            def prep_h(h, lane):
                sfx = str(lane)
                ksr_t = ksp.tile([128, 64], f32, tag="ks_h" + sfx, name="ksr_t")
                ksi_t = ksp.tile([128, 64], f32, tag="ks_h" + sfx, name="ksi_t")
                nc.sync.dma_start(ksr_t[:], ksr_d[h].rearrange("(k1 k2) -> k1 k2", k2=64))
                nc.sync.dma_start(ksi_t[:], ksi_d[h].rearrange("(k1 k2) -> k1 k2", k2=64))
                return (
                    ksr_t[:].unsqueeze(1).broadcast_to([128, GB, 64]),
                    ksi_t[:].unsqueeze(1).broadcast_to([128, GB, 64]),
                    tb["tr"][:].unsqueeze(1).broadcast_to([64, GB, 128]),
                    tb["ti"][:].unsqueeze(1).broadcast_to([64, GB, 128]),
                    tb["tir"][:].unsqueeze(1).broadcast_to([128, GB, 64]),
                    tb["tii"][:].unsqueeze(1).broadcast_to([128, GB, 64]),
                )

            for _rep in range(REPEAT):
                for hp in range(HSH // 2):
                    hA, hB = 2 * hp, 2 * hp + 1
                    kbA = prep_h(hA, 0)
                    kbB = prep_h(hB, 1)
                    for g in range(NG):
                        do_group(hA, g, 0, kbA)
                        do_group(hB, g, 1, kbB)
            for mp in reversed(main_pools):
                mp.__exit__(None, None, None)

    nc.compile()
    return nc


def _get_program():
    key = ("prog", F32R, REPEAT, MIDBUFS, IOBUFS)
    if key not in _CACHE:
        import concourse.bass as bass
        import concourse.tile as tile
        from concourse import mybir, bacc
        _CACHE[key] = _build((bass, tile, mybir, bacc))
    return _CACHE[key]


def kernel(u, A_re, A_im, BC_re, BC_im, D):
    from concourse.bass_utils import run_bass_kernel_spmd

    u = np.ascontiguousarray(u, dtype=np.float32)
    tabs = _tables()
    nc = _get_program()

    in_maps = []
    for c in range(NCORES):
        hs = slice(c * HSH, (c + 1) * HSH)
        m = {
            "u_sh": np.ascontiguousarray(u[:, hs, :]),
            "a_re": np.ascontiguousarray(
                np.concatenate([A_re, A_re]).reshape(2 * P, 1).astype(np.float32)),
            "a_im": np.ascontiguousarray(
                np.concatenate([A_im, A_im]).reshape(2 * P, 1).astype(np.float32)),
            "bct_r": np.ascontiguousarray(BC_re[hs].T.astype(np.float32)),
            "bct_i": np.ascontiguousarray(BC_im[hs].T.astype(np.float32)),
            "bct_i_neg": np.ascontiguousarray(-BC_im[hs].T.astype(np.float32)),
            "d_sh": np.ascontiguousarray(D[hs].reshape(1, HSH).astype(np.float32)),
        }
        m.update(tabs)
        in_maps.append(m)

    res = None
    last_err = None
    for attempt in range(3):
        try:
            res = run_bass_kernel_spmd(nc, in_maps, list(range(NCORES)))
            break
        except Exception as e:  # transient NRT_EXEC_UNIT_UNRECOVERABLE flakes
            last_err = e
            import time as _time
            _time.sleep(2.0)
    if res is None:
        raise last_err
    out = np.concatenate([res.results[c]["y_sh"] for c in range(NCORES)], axis=1)
    return out.astype(np.float32)


if __name__ == "__main__":
    rng = np.random.default_rng(0)
    u = rng.standard_normal((B, H, L), dtype=np.float32)
    A_re = rng.uniform(0.5, 0.99, P).astype(np.float32)
    A_im = rng.uniform(-0.5, 0.5, P).astype(np.float32)
    BC_re = rng.standard_normal((H, P), dtype=np.float32)
    BC_im = rng.standard_normal((H, P), dtype=np.float32)
    D = rng.uniform(0, 1, H).astype(np.float32)
    y = kernel(u=u, A_re=A_re, A_im=A_im, BC_re=BC_re, BC_im=BC_im, D=D)
    print("out", y.shape, y.dtype)

